# revision 27
# baseline (speedup 1.0000x reference)
"""Trainium2 Bass kernel for nn_Net_34763465294339.

Four single-channel VALID convs (K=25/49/97/193, 16 output channels each) on
x[16,1,256,256], each squared + spatially averaged / scale -> stack -> fold
16 channels into 8 by adding halves. Output [16,8,4] f32.

Sharding: data-parallel over batch, 2 images per core, weights replicated.

fp8 DoubleRow row-slab formulation (per conv):
  One matmul per dj0 step per (block-group, image), accumulating in PSUM:
    out[(u,o), (blk,j)] += sum_{(r2,djs),i} W[(r2,djs),i,(u,o)] * X[(r2,djs),i,(blk,j)]
  Contraction partitions (r2, djs): r2 indexes image row PAIRS (DoubleRow
  pair dim i = row parity), djs a dj subgroup. The rhs tile Xg holds raw
  256-byte planar row slices x[b, 2*r2+i+8*(grp*NB+blk), djs:djs+256] --
  full rows, so ONE tile per (group,image) serves every dj0 step via an AP
  column offset of dj0*DJS. Weights W[(r2,djs), i, (u,o)] =
  wq[o, 2*r2+i-u, dj0*DJS+djs] (zero outside ranges) are per-dj0 matrices
  built on host in fp8 (pre-scaled by a per-conv power of two).

  Per-conv normalization (1/(S^2*scale*SW^2)) folds into the ScalarE
  Square activation's input scale. Post: per-(block,image) square+reduce
  into a stage column, ones-matmul fold 16ch->8, DVE column reduce, DMA out.
"""
import numpy as np
import ml_dtypes

import concourse.bass as bass
import concourse.bacc as bacc
import concourse.mybir as mybir
from concourse.tile import TileContext
from concourse.bass_utils import run_bass_kernel_spmd

F32 = mybir.dt.float32
FP8 = mybir.dt.float8e4
NPFP8 = ml_dtypes.float8_e4m3

IMG = 256
NCORES = 8
ROWB = IMG + 1  # planar x padded with one zero row per image

# (K, scale_ref); processing order (heavy convs first). Output feature
# order is fixed by CI below, independent of processing order.
CONVS = [(193, 8.0), (97, 4.0), (49, 2.0), (25, 1.0)]
CI = {25: 0, 49: 1, 97: 2, 193: 3}
# per-conv pow2 weight scale into fp8 sweet spot (w sigmas .05/.02/.01/.005)
SW = {25: 16.0, 49: 64.0, 97: 128.0, 193: 256.0}


def _cfg(K):
    S = IMG - K + 1
    nb = S // 8
    r2 = (K + 7) // 2           # row pairs in the di band
    DJS = max(1, 128 // r2)     # dj subgroups packed into contraction
    steps = -(-K // DJS)        # dj0 steps
    NB = min(512 // S, nb)      # blocks per matmul (psum free cap 512 f32)
    ngrp = -(-nb // NB)
    return S, nb, r2, DJS, steps, NB, ngrp


def _build_w8(wq, K):
    """wq: [16,K,K] f32 already scaled. Returns [r2*DJS, steps*2*8*16] fp8
    with value at ((r2,djs), dj0, i, u, o) = wq[o, 2*r2+i-u, dj0*DJS+djs]."""
    S, nb, r2, DJS, steps, NB, ngrp = _cfg(K)
    M = np.zeros((r2, DJS, steps, 2, 8, 16), dtype=np.float32)
    for p in range(r2):
        for i in range(2):
            for u in range(8):
                di = 2 * p + i - u
                if not (0 <= di < K):
                    continue
                # M[p, djs, dj0, i, u, :] = wq[:, di, dj0*DJS+djs].T
                w_slice = wq[:, di, :]  # [16, K]
                dj = np.arange(steps * DJS)
                valid = dj < K
                dst = np.zeros((steps * DJS, 16), dtype=np.float32)
                dst[valid] = w_slice[:, dj[valid]].T
                M[p, :, :, i, u, :] = dst.reshape(steps, DJS, 16).transpose(1, 0, 2)
    return np.ascontiguousarray(
        M.reshape(r2 * DJS, steps * 2 * 8 * 16)).astype(NPFP8)


def _build_fold():
    F = np.zeros((128, 8), dtype=np.float32)
    for p in range(128):
        F[p, (p % 16) % 8] = 1.0
    return F


def _col_layout(convs):
    col_base = {}
    c = 0
    for (K, scale) in convs:
        nb = (IMG - K + 1) // 8
        for b in range(2):
            col_base[(K, b)] = c
            c += nb
    return col_base, c


def _build_nc(convs, niter=1):
    nc = bacc.Bacc("TRN2", target_bir_lowering=False)
    x = nc.dram_tensor("x", [2, ROWB, IMG], FP8, kind="ExternalInput")
    m_handles = {}
    for (K, scale) in convs:
        S, nb, r2, DJS, steps, NB, ngrp = _cfg(K)
        m_handles[K] = nc.dram_tensor(
            f"m{K}", [r2 * DJS, steps * 256], FP8, kind="ExternalInput")
    fold = nc.dram_tensor("fold", [128, 8], F32, kind="ExternalInput")
    out = nc.dram_tensor("out", [2, 8, 4], F32, kind="ExternalOutput")

    col_base, TOT = _col_layout(convs)

    with TileContext(nc) as tc:
        for _it in range(niter):
            _build_iter(nc, tc, convs, x, m_handles, fold, out,
                        col_base, TOT, _it)
    return nc


def _build_iter(nc, tc, convs, x, m_handles, fold, out, col_base, TOT, it):
    with tc.tile_pool(name=f"consts{it}", bufs=1) as cpool, \
         tc.tile_pool(name=f"xgp{it}", bufs=2) as xpool, \
         tc.tile_pool(name=f"scrp{it}", bufs=4) as spool, \
         tc.tile_pool(name=f"accp{it}", bufs=8, space="PSUM") as ppool:
        m_sb = {}
        for K, h in m_handles.items():
            mt = cpool.tile(list(h.shape), FP8, name=f"msb{K}", tag=f"m{K}")
            nc.sync.dma_start(out=mt[:], in_=h[:])
            m_sb[K] = mt
        fold_sb = cpool.tile([128, 8], F32, name="fold_sb", tag="fold")
        nc.sync.dma_start(out=fold_sb[:], in_=fold[:])
        stage = cpool.tile([128, TOT], F32, name="stage", tag="stage",
                           bufs=2)
        ndma = 0

        for (K, scale) in convs:
            S, nb, r2, DJS, steps, NB, ngrp = _cfg(K)
            s_act = 1.0 / (SW[K] * S * float(np.sqrt(scale)))
            mta = m_sb[K][:]
            pairs = [(grp, b) for grp in range(ngrp) for b in range(2)]
            # dj0-outer over sets of 4 psum groups: consecutive matmuls
            # share the stationary operand, amortizing LDWEIGHTS
            for s0 in range(0, len(pairs), 4):
                gset = pairs[s0:s0 + 4]
                xgs, pss, nbacts = {}, {}, {}
                for (grp, b) in gset:
                    nbact = min(NB, nb - grp * NB)
                    nbacts[(grp, b)] = nbact
                    ntag = sum(1 for g in range(ngrp)
                               if min(NB, nb - g * NB) == nbact)
                    # Xg: partitions (r2,djs); free [i][blk][256B row slice]
                    xg = xpool.tile([r2 * DJS, 2 * nbact * 256], FP8,
                                    name=f"xg{K}_{grp}_{b}",
                                    tag=f"xg{K}_{b}_{nbact}",
                                    bufs=min(8, 2 * ntag + 1))
                    xga = xg[:]
                    # free layout [blk][i][256]: the two row parities of a
                    # pair are adjacent in DRAM, so one contiguous 512-byte
                    # read per block covers both (3-dim AP limit respected);
                    # for DJS==1 the blk dim fits too -> one DMA per tile
                    if DJS == 1:
                        src = bass.AP(
                            x, b * (ROWB * IMG) + grp * NB * 8 * IMG,
                            [[2 * IMG, r2], [8 * IMG, nbact], [1, 512]])
                        dst = bass.AP(xga.tensor, xga.offset,
                                      [xga.ap[0], [512, nbact], [1, 512]])
                        nc.sync.dma_start(out=dst, in_=src)
                        ndma += 1
                    else:
                        for blk in range(nbact):
                            src = bass.AP(
                                x, b * (ROWB * IMG)
                                + (grp * NB + blk) * 8 * IMG,
                                [[2 * IMG, r2], [1, DJS], [1, 512]])
                            dst = bass.AP(
                                xga.tensor, xga.offset + blk * 512,
                                [xga.ap[0], [1, 512]])
                            nc.sync.dma_start(out=dst, in_=src)
                            ndma += 1
                    xgs[(grp, b)] = xga
                    pss[(grp, b)] = ppool.tile(
                        [128, nbact * S], F32,
                        name=f"ps{K}_{grp}_{b}", tag="acc")
                for dj0 in range(steps):
                    lhsT = bass.AP(
                        mta.tensor, mta.offset + dj0 * 256,
                        [mta.ap[0], [128, 2], [1, 128]])
                    for (grp, b) in gset:
                        xga = xgs[(grp, b)]
                        nbact = nbacts[(grp, b)]
                        rhs = bass.AP(
                            xga.tensor, xga.offset + dj0 * DJS,
                            [xga.ap[0], [256, 2],
                             [512, nbact], [1, S]])
                        nc.tensor.matmul(
                            pss[(grp, b)][:], lhsT, rhs,
                            start=(dj0 == 0), stop=(dj0 == steps - 1),
                            perf_mode=mybir.MatmulPerfMode.DoubleRow)
                for (grp, b) in gset:
                    nbact = nbacts[(grp, b)]
                    ps = pss[(grp, b)]
                    # one Square over the whole psum tile on ScalarE, then
                    # per-block column sums on the otherwise-idle DVE
                    scr = spool.tile([128, nbact * S], F32,
                                     name=f"sq{K}_{grp}_{b}", tag="scr")
                    nc.scalar.activation(
                        out=scr[:], in_=ps[:],
                        func=mybir.ActivationFunctionType.Square,
                        scale=float(s_act))
                    for blk in range(nbact):
                        col = col_base[(K, b)] + grp * NB + blk
                        nc.vector.reduce_sum(
                            out=stage[:, col:col + 1],
                            in_=scr[:, blk * S:(blk + 1) * S],
                            axis=mybir.AxisListType.X)

        fold_ps = ppool.tile([8, TOT], F32, name="fold_ps", tag="acc")
        nc.tensor.matmul(fold_ps[:], fold_sb[:], stage[:],
                         start=True, stop=True)
        res = spool.tile([8, 8], F32, name="res", tag="res", bufs=2)
        for (K, scale) in convs:
            ci = CI[K]
            nb = (IMG - K + 1) // 8
            for b in range(2):
                c0 = col_base[(K, b)]
                oc = b * 4 + ci
                nc.vector.reduce_sum(out=res[:8, oc:oc + 1],
                                     in_=fold_ps[:8, c0:c0 + nb],
                                     axis=mybir.AxisListType.X)
        dst = bass.AP(out, 0, [[4, 8], [32, 2], [1, 4]])
        nc.sync.dma_start(out=dst, in_=res[:8, :])


_NC_CACHE = {}


def _get_nc(convs_key, niter=1):
    key = (convs_key, niter)
    if key not in _NC_CACHE:
        nc = _build_nc(list(convs_key), niter=niter)
        nc.compile()
        _NC_CACHE[key] = nc
    return _NC_CACHE[key]


def make_in_maps(inputs, convs=None):
    convs = CONVS if convs is None else convs
    ws = {25: inputs["w0"], 49: inputs["w1"],
          97: inputs["w2"], 193: inputs["w3"]}

    x = np.asarray(inputs["x"], dtype=np.float32).reshape(16, IMG, IMG)

    shared = {}
    for (K, scale) in convs:
        w = np.asarray(ws[K], dtype=np.float32).reshape(16, K, K)
        wq = np.clip(w * SW[K], -240.0, 240.0)
        # quantize weights to fp8 once (matmul sees these exact values)
        wq = wq.astype(NPFP8).astype(np.float32)
        shared[f"m{K}"] = _build_w8(wq, K)
    shared["fold"] = _build_fold()

    in_maps = []
    for c in range(NCORES):
        m = dict(shared)
        xp = np.zeros((2, ROWB, IMG), dtype=NPFP8)
        xp[:, :IMG, :] = np.clip(x[2 * c:2 * c + 2], -240.0, 240.0
                                 ).astype(NPFP8)
        m["x"] = xp
        in_maps.append(m)
    return in_maps


def kernel(x, w0, w1, w2, w3, _convs=None):
    convs = CONVS if _convs is None else _convs
    in_maps = make_in_maps(dict(x=x, w0=w0, w1=w1, w2=w2, w3=w3), convs)
    nc = _get_nc(tuple(convs))
    r = run_bass_kernel_spmd(nc, in_maps, list(range(NCORES)))
    out = np.concatenate([np.asarray(r.results[c]["out"], dtype=np.float32)
                          for c in range(NCORES)], axis=0)
    return out
